# revision 1
# baseline (speedup 1.0000x reference)
"""AFT attention kernel for Trainium2, SPMD across 8 NeuronCores — v2.

Same math/decomposition as baseline kernel.py (bf16 Toeplitz conv on PE),
with:
  - host-side relayout of k/v/q to partition-major bf16 arrays so every
    staging DMA is one fat contiguous run per partition (baseline scattered
    ~100k 256B descriptors);
  - output written in on-chip layout (contiguous DMA) and unpermuted host-side;
  - eviction element-wise work split between DVE and Pool so the DVE is not
    the critical path;
  - same-stationary (e,p) matmul pairs adjacent to help the weight path.

Sharding: channels (192 / 8 cores = 24 per core), zero collectives.
Channel mapping within a core: c(g, j, half) = 4g + 2*half + j.
"""
import numpy as np
from contextlib import ExitStack

import concourse.bass as bass
import concourse.mybir as mybir
import concourse.tile as tile
from concourse import bacc
from concourse.bass_utils import run_bass_kernel_spmd

KS = 31
PAD = KS // 2
H = W = 64
NIMG = 16
C_FULL = 192
NCORES = 8
C_CORE = C_FULL // NCORES      # 24
NGROUPS = C_CORE // 4          # 6
WPAD = W + 2 * PAD             # 94

f32 = mybir.dt.float32
bf16 = mybir.dt.bfloat16

_CACHE = {}


def _build_bmats(w_exp):
    """w_exp: (C, KS, KS) f32 -> per-core B tiles (same as baseline).

    [r, g, 64*half + h_in, j, kx, h_out] = w_exp[c, h_in - h_out + PAD, kx]
    for c = 24*r + 4*g + 2*half + j, zero outside the band.
    """
    hi = np.arange(H)[:, None]
    ho = np.arange(H)[None, :]
    ky = hi - ho + PAD
    valid = (ky >= 0) & (ky < KS)
    kyc = np.clip(ky, 0, KS - 1)
    B = np.where(valid[None, :, None, :],
                 w_exp[:, kyc, :].transpose(0, 1, 3, 2), 0.0)
    B = np.ascontiguousarray(B, dtype=np.float32)
    bm = np.zeros((NCORES, NGROUPS, 128, 2, KS, H), dtype=np.float32)
    for r in range(NCORES):
        for g in range(NGROUPS):
            for j in range(2):
                for half in range(2):
                    c = 24 * r + 4 * g + 2 * half + j
                    bm[r, g, 64 * half:64 * half + 64, j] = B[c]
    return bm


def _emit_group(nc, pools, g, params):
    kv_in, q_in, bmats, out_d, ones1, ones64 = params
    io_pool, xpool, spool, opool, psum_pool, spsum_pool = pools

    # ---- staging loads: single fat DMA per tensor ----
    k_st = io_pool.tile([128, 2, NIMG, W], bf16, tag="k_st")
    v_st = io_pool.tile([128, 2, NIMG, W], bf16, tag="v_st")
    nc.sync.dma_start(k_st[:], kv_in[0, g])
    nc.sync.dma_start(v_st[:], kv_in[1, g])
    b_t = xpool.tile([128, 2, KS, H], bf16, tag="b_t")
    nc.sync.dma_start(b_t[:], bmats[g])

    q_t = []
    for pair in range(2):
        qt = io_pool.tile([128, NIMG, W], bf16, tag=f"q_t{pair}")
        nc.scalar.dma_start(qt[:], q_in[g, pair])
        q_t.append(qt)

    # ---- padded e/p tiles ----
    e_t = xpool.tile([128, 2, NIMG, WPAD], bf16, tag="e_t")
    p_t = xpool.tile([128, 2, NIMG, WPAD], bf16, tag="p_t")
    for t in (e_t, p_t):
        nc.gpsimd.memset(t[:, :, :, 0:PAD], 0.0)
        nc.gpsimd.memset(t[:, :, :, PAD + W:], 0.0)
    for ph in range(2):
        i0 = 8 * ph
        nc.scalar.activation(e_t[:, :, i0:i0 + 8, PAD:PAD + W],
                             k_st[:, :, i0:i0 + 8],
                             mybir.ActivationFunctionType.Exp)
        nc.vector.tensor_mul(p_t[:, :, i0:i0 + 8, PAD:PAD + W],
                             e_t[:, :, i0:i0 + 8, PAD:PAD + W],
                             v_st[:, :, i0:i0 + 8])

    # exp(-q); sigmoid folds into the denominator as (1 + exp(-q)) * den
    eq_t = []
    for pair in range(2):
        eq = spool.tile([128, NIMG, W], f32, tag=f"eq{pair}")
        nc.scalar.activation(eq[:], q_t[pair][:], mybir.ActivationFunctionType.Exp,
                             scale=-1.0)
        eq_t.append(eq)

    # ---- per-(channel, image) sums: w-reduce then ones-matmul over h ----
    red_t = spool.tile([128, 2, 2, NIMG], f32, tag="red_t")
    nc.vector.tensor_reduce(red_t[:, 0], e_t[:, :, :, PAD:PAD + W],
                            mybir.AxisListType.X, mybir.AluOpType.add)
    nc.vector.tensor_reduce(red_t[:, 1], p_t[:, :, :, PAD:PAD + W],
                            mybir.AxisListType.X, mybir.AluOpType.add)

    red_b = spool.tile([128, 2, 2, NIMG], bf16, tag="red_b")
    nc.scalar.copy(red_b.rearrange("p a b c -> p (a b c)"),
                   red_t.rearrange("p a b c -> p (a b c)"))

    # ---- conv accumulation; same-stationary (e,p) pairs adjacent.
    # Phase 0: taps first (PE starts as soon as e/p are ready), then the
    # S-chain matmuls + bias (the sum chain completes in the shadow of the
    # taps).  Phase 1: bias FIRST (s_sb is long ready), taps after — the
    # eviction then waits only on the last tap, shortening the tail. ----
    xt = [e_t, p_t]
    s_sb = None
    for ph in range(2):
        i0 = 8 * ph
        psum_t = {}
        for t in range(2):
            for pair in range(2):
                ps = psum_pool.tile([128, 8, W], f32, tag="ps")
                psum_t[(t, pair)] = ps

        def emit_taps(first, last):
            for kx in range(KS):
                for j in range(2):
                    for half in range(2):
                        for t in range(2):
                            pair, ohalf = half, j
                            ps = psum_t[(t, pair)]
                            p0 = 64 * half
                            nc.tensor.matmul(
                                ps[64 * ohalf:64 * ohalf + 64],
                                b_t[p0:p0 + 64, j, kx, :],
                                xt[t][p0:p0 + 64, j, i0:i0 + 8, kx:kx + W],
                                start=(first and kx == 0),
                                stop=(last and kx == KS - 1))

        def emit_bias(first, last):
            for t in range(2):
                for pair in range(2):
                    ps = psum_t[(t, pair)]
                    for half in range(2):
                        sp0 = 64 * pair
                        rhs = (s_sb[sp0:sp0 + 64, t, half, i0:i0 + 8]
                               .broadcast_to((64, 8, W)))
                        nc.tensor.matmul(ps[64 * half:64 * half + 64],
                                         ones64[sp0:sp0 + 64, :], rhs,
                                         start=first, stop=last)

        if ph == 0:
            emit_taps(first=True, last=False)
            s_ps = spsum_pool.tile([128, 64], f32, tag="sps")
            for half in range(2):
                p0 = 64 * half
                nc.tensor.matmul(s_ps[p0:p0 + 64, :], ones1[p0:p0 + 64, :],
                                 red_b[p0:p0 + 64].rearrange("p a b c -> p (a b c)"),
                                 start=True, stop=True)
            s_sb = spool.tile([128, 2, 2, NIMG], bf16, tag="s_sb")
            nc.scalar.copy(s_sb.rearrange("p a b c -> p (a b c)"), s_ps[:])
            emit_bias(first=False, last=True)
        else:
            emit_bias(first=True, last=False)
            emit_taps(first=False, last=True)

        # ---- eviction: out = psum_p * recip((1 + exp(-q)) * psum_e) ----
        for pair in range(2):
            den = opool.tile([128, 8, W], f32, tag=f"den{pair}")
            nc.vector.scalar_tensor_tensor(
                den[:], eq_t[pair][:, i0:i0 + 8, :], 1.0, psum_t[(0, pair)][:],
                op0=mybir.AluOpType.add, op1=mybir.AluOpType.mult)
            nc.vector.reciprocal_approx_fast(
                out=den.rearrange("p a b -> p (a b)"),
                in_=den.rearrange("p a b -> p (a b)"))
            o_t = opool.tile([128, 8, W], f32, tag=f"o_t{pair}")
            nc.vector.tensor_mul(o_t[:], psum_t[(1, pair)][:], den[:])
            nc.scalar.dma_start(out_d[g, pair, ph], o_t[:])


def _build_nc():
    nc = bacc.Bacc("TRN2", target_bir_lowering=False, debug=False,
                   num_devices=NCORES)
    # [k/v, g, (half,h)=128, j, img, w] bf16
    kv_in = nc.declare_dram_parameter("kv_in", [2, NGROUPS, 128, 2, NIMG, W],
                                      bf16, isOutput=False)
    # [g, pair, (half,h)=128, img, w] bf16
    q_in = nc.declare_dram_parameter("q_in", [NGROUPS, 2, 128, NIMG, W],
                                     bf16, isOutput=False)
    bmats = nc.declare_dram_parameter("bmats", [NGROUPS, 128, 2, KS, H],
                                      bf16, isOutput=False)
    # [g, pair, ph, (j,h)=128, img8, w] f32
    out_d = nc.declare_dram_parameter("out", [NGROUPS, 2, 2, 128, 8, W],
                                      f32, isOutput=True)

    with tile.TileContext(nc) as tc:
        with ExitStack() as ctx:
            io_pool = ctx.enter_context(tc.tile_pool(name="io", bufs=3))
            xpool = ctx.enter_context(tc.tile_pool(name="x", bufs=3))
            spool = ctx.enter_context(tc.tile_pool(name="s", bufs=2))
            opool = ctx.enter_context(tc.tile_pool(name="o", bufs=2))
            cpool = ctx.enter_context(tc.tile_pool(name="c", bufs=1))
            psum_pool = ctx.enter_context(tc.tile_pool(name="psum", bufs=6, space="PSUM"))
            spsum_pool = ctx.enter_context(tc.tile_pool(name="spsum", bufs=2, space="PSUM"))

            ones1 = cpool.tile([128, 64], bf16)
            nc.vector.memset(ones1[:], 1.0)
            ones64 = cpool.tile([128, 64], bf16)
            nc.vector.memset(ones64[:], 1.0 / 64.0)

            params = (kv_in, q_in, bmats, out_d, ones1, ones64)
            pools = (io_pool, xpool, spool, opool, psum_pool, spsum_pool)
            for g in range(NGROUPS):
                _emit_group(nc, pools, g, params)
    nc.compile()
    return nc


def _get_nc():
    if "nc" not in _CACHE:
        _CACHE["nc"] = _build_nc()
    return _CACHE["nc"]


def run(qkv, weights, trace=False):
    import ml_dtypes
    qkv = np.asarray(qkv, dtype=np.float32)
    weights = np.asarray(weights, dtype=np.float32)
    assert qkv.shape == (NIMG, 3 * C_FULL, H * W), qkv.shape
    assert weights.shape == (C_FULL, 1, KS, KS), weights.shape

    w_exp = np.expm1(weights[:, 0].astype(np.float64)).astype(np.float32)
    bm = _build_bmats(w_exp).astype(ml_dtypes.bfloat16)

    q = qkv[:, :C_FULL].reshape(NIMG, C_FULL, H, W)
    k = qkv[:, C_FULL:2 * C_FULL].reshape(NIMG, C_FULL, H, W)
    v = qkv[:, 2 * C_FULL:].reshape(NIMG, C_FULL, H, W)

    in_maps = []
    for r in range(NCORES):
        cs = slice(24 * r, 24 * r + 24)
        # c = 4g + 2*half + j  ->  index split (g, half, j)
        # kv layout [g, (half,h)=128, j, i, w]
        kr = k[:, cs].reshape(NIMG, NGROUPS, 2, 2, H, W)   # i,g,half,j,h,w
        vr = v[:, cs].reshape(NIMG, NGROUPS, 2, 2, H, W)
        kv = np.empty((2, NGROUPS, 128, 2, NIMG, W), dtype=ml_dtypes.bfloat16)
        kv[0] = kr.transpose(1, 2, 4, 3, 0, 5).reshape(NGROUPS, 128, 2, NIMG, W)
        kv[1] = vr.transpose(1, 2, 4, 3, 0, 5).reshape(NGROUPS, 128, 2, NIMG, W)
        # q layout [g, pair, (half,h)=128, i, w]; c = 4g + 2*pair + half
        qr = q[:, cs].reshape(NIMG, NGROUPS, 2, 2, H, W)   # i,g,pair,half,h,w
        qh = np.ascontiguousarray(
            qr.transpose(1, 2, 3, 4, 0, 5).reshape(NGROUPS, 2, 128, NIMG, W)
        ).astype(ml_dtypes.bfloat16)
        in_maps.append({"kv_in": kv, "q_in": qh, "bmats": bm[r]})

    nc = _get_nc()
    res = run_bass_kernel_spmd(nc, in_maps, core_ids=list(range(NCORES)),
                               trace=trace)
    out = np.empty((NIMG, C_FULL, H * W), dtype=np.float32)
    for r in range(NCORES):
        # [g, pair, ph, (j,h)=128, i8, w] -> out[8ph+i, 4g+2pair+j, h, w]
        o = np.asarray(res.results[r]["out"], dtype=np.float32)
        o = o.reshape(NGROUPS, 2, 2, 2, H, 8, W)          # g,pair,ph,j,h,i,w
        o = o.transpose(2, 5, 0, 1, 3, 4, 6)              # ph,i,g,pair,j,h,w
        out[:, 24 * r:24 * r + 24, :] = o.reshape(NIMG, C_CORE, H * W)
    return out, res


def kernel(qkv, weights, H=None, W=None, **_unused):
    out, _ = run(qkv, weights)
    return out



# revision 6
# speedup vs baseline: 1.1098x; 1.1098x over previous
"""AFT attention kernel for Trainium2, SPMD across 8 NeuronCores — v4.

Same bf16 Toeplitz-conv decomposition as v2 (per-kx taps, Toeplitz band
over h on the PE, 4-way quadrant concurrency), plus:
  - clipped tap windows: instead of zero-padding e/p to width 94 and
    streaming 64 columns per tap, each tap streams only its valid
    64-|kx-15| output columns via a shifted PSUM write window
    (~12% less PE stream time, no pads, no memsets);
  - phase 0 opens each PSUM bank with the full-width kx=15 tap
    (start=True covers every cell), phase 1 opens with the bias matmul;
  - bf16 output DMA (half the store traffic);
  - dma_start issue moved off the Scalar queue (inputs on Sync,
    outputs on GpSimd).

Sharding: channels (192 / 8 cores = 24 per core), zero collectives.
Channel mapping within a core: c(g, j, half) = 4g + 2*half + j.
"""
import numpy as np
from contextlib import ExitStack

import concourse.bass as bass
import concourse.mybir as mybir
import concourse.tile as tile
from concourse import bacc
from concourse.bass_utils import run_bass_kernel_spmd

KS = 31
PAD = KS // 2
H = W = 64
NIMG = 16
C_FULL = 192
NCORES = 8
C_CORE = C_FULL // NCORES      # 24
NGROUPS = C_CORE // 4          # 6

f32 = mybir.dt.float32
bf16 = mybir.dt.bfloat16

# kx=15 first (full window, opens the PSUM banks), then the rest
KX_ORDER = [15] + [kx for kx in range(KS) if kx != 15]

_CACHE = {}


def _build_bmats(w_exp):
    """w_exp: (C, KS, KS) f32 -> per-core B tiles.

    [r, g, 64*half + h_in, j, kx, h_out] = w_exp[c, h_in - h_out + PAD, kx]
    for c = 24*r + 4*g + 2*half + j, zero outside the band.
    """
    hi = np.arange(H)[:, None]
    ho = np.arange(H)[None, :]
    ky = hi - ho + PAD
    valid = (ky >= 0) & (ky < KS)
    kyc = np.clip(ky, 0, KS - 1)
    B = np.where(valid[None, :, None, :],
                 w_exp[:, kyc, :].transpose(0, 1, 3, 2), 0.0)
    B = np.ascontiguousarray(B, dtype=np.float32)
    bm = np.zeros((NCORES, NGROUPS, 128, 2, KS, H), dtype=np.float32)
    for r in range(NCORES):
        for g in range(NGROUPS):
            for j in range(2):
                for half in range(2):
                    c = 24 * r + 4 * g + 2 * half + j
                    bm[r, g, 64 * half:64 * half + 64, j] = B[c]
    return bm


def _emit_group(nc, pools, g, params):
    kv_in, q_in, bmats, out_d, ones1, ones64 = params
    io_pool, xpool, spool, opool, psum_pool, spsum_pool = pools

    # ---- staging loads: single fat DMA per tensor ----
    k_st = io_pool.tile([128, 2, NIMG, W], bf16, tag="k_st")
    v_st = io_pool.tile([128, 2, NIMG, W], bf16, tag="v_st")
    nc.sync.dma_start(k_st[:], kv_in[0, g])
    nc.sync.dma_start(v_st[:], kv_in[1, g])
    b_t = xpool.tile([128, 2, KS, H], bf16, tag="b_t")
    nc.sync.dma_start(b_t[:], bmats[g])

    q_t = []
    for pair in range(2):
        qt = io_pool.tile([128, NIMG, W], bf16, tag=f"q_t{pair}")
        nc.sync.dma_start(qt[:], q_in[g, pair])
        q_t.append(qt)

    # ---- e/p tiles (no padding) ----
    e_t = xpool.tile([128, 2, NIMG, W], bf16, tag="e_t")
    p_t = xpool.tile([128, 2, NIMG, W], bf16, tag="p_t")
    for ph in range(2):
        i0 = 8 * ph
        nc.scalar.activation(e_t[:, :, i0:i0 + 8, :], k_st[:, :, i0:i0 + 8],
                             mybir.ActivationFunctionType.Exp)
        nc.vector.tensor_mul(p_t[:, :, i0:i0 + 8, :],
                             e_t[:, :, i0:i0 + 8, :],
                             v_st[:, :, i0:i0 + 8])

    # exp(-q); sigmoid folds into the denominator as (1 + exp(-q)) * den
    eq_t = []
    for pair in range(2):
        eq = spool.tile([128, NIMG, W], f32, tag=f"eq{pair}")
        nc.scalar.activation(eq[:], q_t[pair][:], mybir.ActivationFunctionType.Exp,
                             scale=-1.0)
        eq_t.append(eq)

    # ---- per-(channel, image) sums: w-reduce then ones-matmul over h ----
    red_t = spool.tile([128, 2, 2, NIMG], f32, tag="red_t")
    nc.vector.tensor_reduce(red_t[:, 0], e_t[:],
                            mybir.AxisListType.X, mybir.AluOpType.add)
    nc.vector.tensor_reduce(red_t[:, 1], p_t[:],
                            mybir.AxisListType.X, mybir.AluOpType.add)

    red_b = spool.tile([128, 2, 2, NIMG], bf16, tag="red_b")
    nc.scalar.copy(red_b.rearrange("p a b c -> p (a b c)"),
                   red_t.rearrange("p a b c -> p (a b c)"))

    # ---- conv accumulation with clipped tap windows ----
    xt = [e_t, p_t]
    s_sb = None
    for ph in range(2):
        i0 = 8 * ph
        psum_t = {}
        for t in range(2):
            for pair in range(2):
                pst = psum_pool.tile([128, 8, W], f32, tag="ps")
                psum_t[(t, pair)] = pst

        def emit_taps(first, last):
            for kx in KX_ORDER:
                s = kx - PAD
                w0 = max(0, -s)      # first valid output column
                wd = W - abs(s)      # window width
                c0 = w0 + s          # rhs column start (= max(s, 0))
                for j in range(2):
                    for half in range(2):
                        for t in range(2):
                            pair, ohalf = half, j
                            ps = psum_t[(t, pair)]
                            p0 = 64 * half
                            nc.tensor.matmul(
                                ps[64 * ohalf:64 * ohalf + 64, :, w0:w0 + wd],
                                b_t[p0:p0 + 64, j, kx, :],
                                xt[t][p0:p0 + 64, j, i0:i0 + 8, c0:c0 + wd],
                                start=(first and kx == 15),
                                stop=(last and kx == KX_ORDER[-1]))

        def emit_bias(first, last):
            for t in range(2):
                for pair in range(2):
                    ps = psum_t[(t, pair)]
                    for half in range(2):
                        sp0 = 64 * pair
                        rhs = (s_sb[sp0:sp0 + 64, t, half, i0:i0 + 8]
                               .broadcast_to((64, 8, W)))
                        nc.tensor.matmul(ps[64 * half:64 * half + 64], ones64[sp0:sp0 + 64, :], rhs,
                                         start=first, stop=last)

        if ph == 0:
            emit_taps(first=True, last=False)
            s_ps = spsum_pool.tile([128, 64], f32, tag="sps")
            for half in range(2):
                p0 = 64 * half
                nc.tensor.matmul(s_ps[p0:p0 + 64, :], ones1[p0:p0 + 64, :],
                                 red_b[p0:p0 + 64].rearrange("p a b c -> p (a b c)"),
                                 start=True, stop=True)
            s_sb = spool.tile([128, 2, 2, NIMG], bf16, tag="s_sb")
            nc.scalar.copy(s_sb.rearrange("p a b c -> p (a b c)"), s_ps[:])
            emit_bias(first=False, last=True)
        else:
            emit_bias(first=True, last=False)
            emit_taps(first=False, last=True)

        # ---- eviction: out = psum_p * recip((1 + exp(-q)) * psum_e) ----
        for pair in range(2):
            den = opool.tile([128, 8, W], f32, tag=f"den{pair}")
            nc.vector.scalar_tensor_tensor(
                den[:], eq_t[pair][:, i0:i0 + 8, :], 1.0, psum_t[(0, pair)][:],
                op0=mybir.AluOpType.add, op1=mybir.AluOpType.mult)
            nc.vector.reciprocal_approx_fast(
                out=den.rearrange("p a b -> p (a b)"),
                in_=den.rearrange("p a b -> p (a b)"))
            o_t = opool.tile([128, 8, W], bf16, tag=f"o_t{pair}")
            nc.vector.tensor_mul(o_t[:], psum_t[(1, pair)][:], den[:])
            nc.gpsimd.dma_start(out_d[g, pair, ph], o_t[:])


def _build_nc():
    nc = bacc.Bacc("TRN2", target_bir_lowering=False, debug=False,
                   num_devices=NCORES)
    # [k/v, g, (half,h)=128, j, img, w] bf16
    kv_in = nc.declare_dram_parameter("kv_in", [2, NGROUPS, 128, 2, NIMG, W],
                                      bf16, isOutput=False)
    # [g, pair, (half,h)=128, img, w] bf16
    q_in = nc.declare_dram_parameter("q_in", [NGROUPS, 2, 128, NIMG, W],
                                     bf16, isOutput=False)
    bmats = nc.declare_dram_parameter("bmats", [NGROUPS, 128, 2, KS, H],
                                      bf16, isOutput=False)
    # [g, pair, ph, (j,h)=128, img8, w] bf16
    out_d = nc.declare_dram_parameter("out", [NGROUPS, 2, 2, 128, 8, W],
                                      bf16, isOutput=True)

    with tile.TileContext(nc) as tc:
        with ExitStack() as ctx:
            io_pool = ctx.enter_context(tc.tile_pool(name="io", bufs=3))
            xpool = ctx.enter_context(tc.tile_pool(name="x", bufs=3))
            spool = ctx.enter_context(tc.tile_pool(name="s", bufs=2))
            opool = ctx.enter_context(tc.tile_pool(name="o", bufs=2))
            cpool = ctx.enter_context(tc.tile_pool(name="c", bufs=1))
            psum_pool = ctx.enter_context(tc.tile_pool(name="psum", bufs=6, space="PSUM"))
            spsum_pool = ctx.enter_context(tc.tile_pool(name="spsum", bufs=2, space="PSUM"))

            ones1 = cpool.tile([128, 64], bf16)
            nc.vector.memset(ones1[:], 1.0)
            ones64 = cpool.tile([128, 64], bf16)
            nc.vector.memset(ones64[:], 1.0 / 64.0)

            params = (kv_in, q_in, bmats, out_d, ones1, ones64)
            pools = (io_pool, xpool, spool, opool, psum_pool, spsum_pool)
            for g in range(NGROUPS):
                _emit_group(nc, pools, g, params)
    nc.compile()
    return nc


def _get_nc():
    if "nc" not in _CACHE:
        _CACHE["nc"] = _build_nc()
    return _CACHE["nc"]


def run(qkv, weights, trace=False):
    import ml_dtypes
    qkv = np.asarray(qkv, dtype=np.float32)
    weights = np.asarray(weights, dtype=np.float32)
    assert qkv.shape == (NIMG, 3 * C_FULL, H * W), qkv.shape
    assert weights.shape == (C_FULL, 1, KS, KS), weights.shape

    w_exp = np.expm1(weights[:, 0].astype(np.float64)).astype(np.float32)
    bm = _build_bmats(w_exp).astype(ml_dtypes.bfloat16)

    q = qkv[:, :C_FULL].reshape(NIMG, C_FULL, H, W)
    k = qkv[:, C_FULL:2 * C_FULL].reshape(NIMG, C_FULL, H, W)
    v = qkv[:, 2 * C_FULL:].reshape(NIMG, C_FULL, H, W)

    in_maps = []
    for r in range(NCORES):
        cs = slice(24 * r, 24 * r + 24)
        # c = 4g + 2*half + j  ->  index split (g, half, j)
        # kv layout [g, (half,h)=128, j, i, w]
        kr = k[:, cs].reshape(NIMG, NGROUPS, 2, 2, H, W)   # i,g,half,j,h,w
        vr = v[:, cs].reshape(NIMG, NGROUPS, 2, 2, H, W)
        kv = np.empty((2, NGROUPS, 128, 2, NIMG, W), dtype=ml_dtypes.bfloat16)
        kv[0] = kr.transpose(1, 2, 4, 3, 0, 5).reshape(NGROUPS, 128, 2, NIMG, W)
        kv[1] = vr.transpose(1, 2, 4, 3, 0, 5).reshape(NGROUPS, 128, 2, NIMG, W)
        # q layout [g, pair, (half,h)=128, i, w]; c = 4g + 2*pair + half
        qr = q[:, cs].reshape(NIMG, NGROUPS, 2, 2, H, W)   # i,g,pair,half,h,w
        qh = np.ascontiguousarray(
            qr.transpose(1, 2, 3, 4, 0, 5).reshape(NGROUPS, 2, 128, NIMG, W)
        ).astype(ml_dtypes.bfloat16)
        in_maps.append({"kv_in": kv, "q_in": qh, "bmats": bm[r]})

    nc = _get_nc()
    res = run_bass_kernel_spmd(nc, in_maps, core_ids=list(range(NCORES)),
                               trace=trace)
    out = np.empty((NIMG, C_FULL, H * W), dtype=np.float32)
    for r in range(NCORES):
        # [g, pair, ph, (j,h)=128, i8, w] -> out[8ph+i, 4g+2pair+j, h, w]
        o = np.asarray(res.results[r]["out"], dtype=np.float32)
        o = o.reshape(NGROUPS, 2, 2, 2, H, 8, W)          # g,pair,ph,j,h,i,w
        o = o.transpose(2, 5, 0, 1, 3, 4, 6)              # ph,i,g,pair,j,h,w
        out[:, 24 * r:24 * r + 24, :] = o.reshape(NIMG, C_CORE, H * W)
    return out, res


def kernel(qkv, weights, H=None, W=None, **_unused):
    out, _ = run(qkv, weights)
    return out
